# revision 1
# baseline (speedup 1.0000x reference)
"""Masked attention on 8 TRN2 NeuronCores — pure data-parallel over batch.

Full inputs:  q,k,v (16,2048,128) f32, mask (16,2048,2048) bool.
Output:       (16,2048,128) f32.

Per core (2 batches). Scores are computed transposed S^T[k,q] = K·Q^T in bf16
on the PE so the AV contraction lands on the partition axis. The exp+mask
elementwise pass (8.4M elems/core, the classic ACT bottleneck) is split across
engines by k-block pair, tuned against the CoreSim cost model:

  - SCHR pairs: a pair-averaged Schraudolph exp approximation fused with the
    mask, no ACT involvement: i1 = trunc(s*a + mb) with a = 128/(ln2*sqrt(D))
    and mb[k,q] int16 host-built (live: 16256-134, folding the pair-average
    halving and the mean log error of the approximation; masked: 2048 so the
    bitcast lands at ~1e-34 ~ 0). i1's bit pattern read as bf16 is e^x*g(f)
    with a +/-3% periodic ripple g; i2 = i1+64 samples g a half-octave later,
    and y = bitcast(i2)/sqrt(2) + bitcast(i1) cancels the ripple's odd
    harmonics (residual ~0.3%, at the bf16 noise floor). The first op
    (scalar_tensor_tensor from f32 PSUM) runs on Pool (flat-rate engine);
    the +64 / scale / add run on DVE in 4x/2x modes.
  - E16 pairs: exact ACT exp from PSUM, then DVE tensor_mul with an int16
    0/1 not-mask (2-byte operands keep the DVE in 2x mode).
  - EU8P pairs: exact ACT exp, mask-mul with uint8 not-mask on Pool via the
    scalar_tensor_tensor form (Pool charges flat cycle rate).

Pair order interleaves SCHR between exp pairs so the ACT queue never bunches,
and the last pair is E16 (shortest QK->attn latency) to minimize the AV
drain tail. AV accumulates [q,129] per slot with a ones-column in va giving
the softmax denominator; normalization is DVE reciprocal + ACT/Pool scale,
stored as [q, d] rows directly from DVE's DMA queue (no transposes).
"""

import numpy as np
import ml_dtypes

B, S, D = 16, 2048, 128
N_CORES = 8
BPC = B // N_CORES   # batches per core
P = 128              # partitions
QW = 512             # q-tile width
NQT = S // QW        # q-tiles per batch
KB = S // P          # k-blocks per batch
NPAIR = KB // 2      # k-block pairs per q-tile pass
PD = 5               # AV matmuls pipelined this many pairs behind elementwise
SPSUM_BUFS = 3       # scores double/triple buffering (2 banks each)
AV_BUFS = 2          # AV accumulator tiles (1 bank each)
NORM_SL_ENG = ["DVE", "DVE", "DVE", "DVE"]  # per-slot scale engine (PSUM: no Pool)
STORE_ENG = "SP"     # which engine queue issues the out store
M8_LD = "SP"         # engine for merged u8 mask loads (SP | POOL)
M16_LD = ["SP", "SP", "SP"]  # engine per E16 mask pair position
ATT_BUFS = 8
ATTE_BUFS = 9
WARM_N = 14

# Per-pair elementwise path (8 entries, k-block pairs 0..7):
#   SCHR: pair-averaged Schraudolph, Pool stt + DVE tail (m16 bias rows)
#   E16:  ACT exp + DVE tensor_mul (m16 0/1 int16 rows)
#   EU8P: ACT exp + Pool stt-mul (m8 uint8 rows)
PATHS = ["SCHR", "SCHR", "E16", "EU8P", "EU8P", "EU8P", "EU8P", "E16"]
# m16 (int16 0/1) serves E16 pairs; m8 (uint8) serves SCHR pairs (bias
# codes {252 live, 24 dead}, decoded by x63.976 in the stt) and EU8P pairs
# ({1, 0} multiplier)
M16_PAIRS = [i for i, p in enumerate(PATHS) if p == "E16"]
M8_PAIRS = [i for i, p in enumerate(PATHS) if p in ("SCHR", "EU8P")]
M16_POS = {pr: i for i, pr in enumerate(M16_PAIRS)}
M8_POS = {pr: i for i, pr in enumerate(M8_PAIRS)}
SCHR_BIAS_LIVE = 16256.0 - 134.0
SCHR_M_LIVE = 252
SCHR_M_DEAD = 24
SCHR_MSCALE = SCHR_BIAS_LIVE / SCHR_M_LIVE  # 63.976...: m*this = bias

_NC = None
LAST_RESULT = None   # BassKernelResults of the most recent run (for profiling)


def _build_nc(bpc=BPC, s=S):
    import concourse.bacc as bacc
    import concourse.tile as tile
    from concourse import mybir

    bf16 = mybir.dt.bfloat16
    f32 = mybir.dt.float32
    i16 = mybir.dt.int16
    u8 = mybir.dt.uint8

    # qT is host-pre-scaled by a = 128/(ln2*sqrt(D)), so PSUM scores are
    # already in Schraudolph units; exact exp just rescales by ln2/128
    escale = float(np.log(2.0) / P)
    invs2 = float(1.0 / np.sqrt(2.0))
    mscale = float(SCHR_MSCALE)

    n16 = len(M16_PAIRS) * 2     # k-blocks in m16
    n8 = len(M8_PAIRS) * 2       # k-blocks in m8

    nc = bacc.Bacc()
    qT = nc.declare_dram_parameter("qT", [bpc, P, s], bf16, isOutput=False)
    kT = nc.declare_dram_parameter("kT", [bpc, P, s], bf16, isOutput=False)
    # va host-packed as [p, kb*(D+1)]: row p holds v[kb*128+p, :]+[1] per kb
    va = nc.declare_dram_parameter(
        "va", [bpc, P, KB * (D + 1)], bf16, isOutput=False
    )
    # m16[b, i*2P+p, q] int16, i indexes M16_PAIRS: Schraudolph bias rows for
    # SCHR pairs, 0/1 not-mask rows for E16 pairs
    m16 = nc.declare_dram_parameter("m16", [bpc, n16 * P, s], i16, isOutput=False)
    # m8: uint8 not-mask rows for EU8P pairs (indexed by M8_PAIRS position)
    m8 = nc.declare_dram_parameter("m8", [bpc, n8 * P, s], u8, isOutput=False)
    # out packed [b, qt, p, sl*D+d] so each partition row is one contiguous
    # 1KB chunk (avoids the <512B DMA latency penalty); host unpermutes
    out = nc.declare_dram_parameter(
        "out", [bpc, NQT, P, 4 * D], bf16, isOutput=True
    )

    with tile.TileContext(nc) as tc:
        with (
            tc.tile_pool(name="qk", bufs=2) as qkp,
            tc.tile_pool(name="vp", bufs=2) as vp,
            tc.tile_pool(name="m16p", bufs=3) as m16p,
            tc.tile_pool(name="m8p", bufs=3) as m8p,
            tc.tile_pool(name="attn", bufs=ATT_BUFS) as attnp,
            tc.tile_pool(name="attne", bufs=ATTE_BUFS) as attnep,
            tc.tile_pool(name="outp", bufs=3) as outp,
            tc.tile_pool(name="const", bufs=1) as constp,
            tc.tile_pool(name="rp", bufs=12) as rp,
            tc.tile_pool(name="spsum", bufs=SPSUM_BUFS, space="PSUM") as spsum,
            tc.tile_pool(name="avpsum", bufs=AV_BUFS, space="PSUM") as avpsum,
        ):
            # ACT warm-up: load the Exp table (~1.3us) during initial DMAs
            warm = constp.tile([P, 128], bf16, name="warm", tag="warm")
            nc.vector.memset(warm[:], 0.0)
            nc.scalar.activation(
                warm[:, 0:1], warm[:, 0:1], mybir.ActivationFunctionType.Exp
            )
            # PE warm-up burst: p-state ramps to full clock with sustained
            # busy; burn the initial DMA-wait window on dummy matmuls
            wp = avpsum.tile([P, 2, D + 1], f32, name="warm_mm", tag="av")
            for _ in range(WARM_N):
                nc.tensor.matmul(
                    wp[:, 0, 0:P], lhsT=warm[:], rhs=warm[:],
                    start=True, stop=True,
                )

            def emit_norm(job):
                av_t, o_s, b_, qt_ = job
                for sl in range(4):
                    avap = av_t[sl // 2][:, sl % 2, :]
                    recip = rp.tile([P, 1], f32, name="recip", tag="recip")
                    nc.vector.reciprocal(recip[:], avap[:, D:D + 1])
                    eng_n = NORM_SL_ENG[sl]
                    if eng_n == "ACT":
                        nc.scalar.activation(
                            o_s[:, sl, :], avap[:, 0:D],
                            mybir.ActivationFunctionType.Copy,
                            scale=recip[:],
                        )
                    elif eng_n == "POOL":
                        nc.gpsimd.tensor_scalar(
                            out=o_s[:, sl, :], in0=avap[:, 0:D],
                            scalar1=recip[:], scalar2=None,
                            op0=mybir.AluOpType.mult,
                        )
                    else:
                        nc.vector.tensor_scalar_mul(
                            o_s[:, sl, :], avap[:, 0:D], recip[:]
                        )
                seng = nc.scalar if STORE_ENG == "ACT" else nc.sync
                seng.dma_start(
                    out=out[b_, qt_, :, :].rearrange(
                        "p (sl d) -> p sl d", d=D
                    ),
                    in_=o_s[:, :, :],
                )

            for b in range(bpc):
                # kT in ascending chunks: kb 0-1, 2-3, 4-7, 8-11, 12-15
                KCW = [2 * P, 2 * P, 4 * P, 4 * P, 4 * P]
                KCO = [0, 2 * P, 4 * P, 8 * P, 12 * P]
                kt_ch = [
                    qkp.tile([P, w], bf16, name="kt_ch", tag=f"kt{i}", bufs=2)
                    for i, w in enumerate(KCW)
                ]
                qt_ch = [
                    qkp.tile([P, QW], bf16, name="qt_ch", tag="qt", bufs=NQT + 2)
                    for _ in range(NQT)
                ]
                va_s = vp.tile([P, KB, D + 1], bf16, name="va_s", tag="va")
                vr = va[b, :, :].rearrange("p (kb d) -> p kb d", d=D + 1)
                m16_t = [
                    m16p.tile([P, n16, QW], i16, name="m16_t", tag="m16",
                              bufs=NQT + 2)
                    for _ in range(NQT)
                ]
                m8_t = [
                    m8p.tile([P, n8, QW], u8, name="m8_t", tag="m8",
                             bufs=NQT + 2)
                    for _ in range(NQT)
                ]

                m16r = m16[b, :, :].rearrange("(kb p) q -> p kb q", p=P)
                m8r = m8[b, :, :].rearrange("(kb p) q -> p kb q", p=P)

                def _ldk(i):
                    nc.sync.dma_start(
                        out=kt_ch[i][:], in_=kT[b, :, KCO[i]:KCO[i] + KCW[i]]
                    )

                def _ldq(qt):
                    nc.sync.dma_start(
                        out=qt_ch[qt][:], in_=qT[b, :, qt * QW:(qt + 1) * QW]
                    )

                def _ldm16(qt, pos):
                    # one pair-row-block (2 k-blocks) of m16 for this q-tile
                    eng = {"SP": nc.sync, "ACT": nc.scalar}[M16_LD[pos]]
                    eng.dma_start(
                        out=m16_t[qt][:, 2 * pos:2 * pos + 2, :],
                        in_=m16r[:, 2 * pos:2 * pos + 2, qt * QW:(qt + 1) * QW],
                    )

                def _ldm8(qt, pos):
                    nc.sync.dma_start(
                        out=m8_t[qt][:, 2 * pos:2 * pos + 2, :],
                        in_=m8r[:, 2 * pos:2 * pos + 2, qt * QW:(qt + 1) * QW],
                    )

                # issue order: deadline-sorted for batch-0 q-tile-0: QK(t)
                # needs kt chunks in order; SCHR pair-0 mask feeds the first
                # attn tile; va needed by AV(t0) at ~t+3us; the rest stream
                def _ldmask(qt, t):
                    if PATHS[t] == "E16":
                        _ldm16(qt, M16_POS[t])
                    else:
                        _ldm8(qt, M8_POS[t])

                def _ldm8_all(qt):
                    # one DMA for all u8 pair rows of this q-tile: the 500ns
                    # descriptor-gen floor makes per-pair loads pay more
                    eng = nc.sync if M8_LD == "SP" else nc.gpsimd
                    eng.dma_start(
                        out=m8_t[qt][:],
                        in_=m8r[:, :, qt * QW:(qt + 1) * QW],
                    )

                def _ldm16_all(qt):
                    for pos in range(len(M16_PAIRS)):
                        _ldm16(qt, pos)

                # deadline-interleaved: pair-t masks early, kt chunks at QK
                # deadlines, va halves before the first AVs need them
                _ldk(0)
                _ldq(0)
                _ldk(1)
                _ldmask(0, 0)
                _ldmask(0, 1)
                _ldk(2)
                nc.sync.dma_start(out=va_s[:, 0:8, :], in_=vr[:, 0:8, :])
                _ldmask(0, 2)
                _ldk(3)
                _ldmask(0, 3)
                _ldk(4)
                nc.sync.dma_start(out=va_s[:, 8:KB, :], in_=vr[:, 8:KB, :])
                for t in range(4, NPAIR):
                    _ldmask(0, t)
                for qt in range(1, NQT):
                    _ldq(qt)
                    for t in range(NPAIR):
                        _ldmask(qt, t)

                for qt in range(NQT):
                    av_t = [
                        avpsum.tile([P, 2, D + 1], f32, name="av", tag="av")
                        for _ in range(2)
                    ]
                    o_s = outp.tile([P, 4, D], bf16, name="o_s", tag="o_s")
                    atts = [None] * NPAIR
                    for t in range(NPAIR + PD):
                        if t >= PD:
                            tp = t - PD
                            for h in range(2):
                                kb = 2 * tp + h
                                for sl in range(4):
                                    nc.tensor.matmul(
                                        av_t[sl // 2][:, sl % 2, :],
                                        lhsT=atts[tp][:, h, sl * P:(sl + 1) * P],
                                        rhs=va_s[:, kb, :],
                                        start=(kb == 0 and sl % 2 == 0),
                                        stop=(kb == KB - 1 and sl % 2 == 1),
                                    )
                        if t < NPAIR:
                            s_ps = spsum.tile(
                                [P, 2, QW], f32, name="s_ps", tag="s"
                            )
                            for h in range(2):
                                kb = 2 * t + h
                                kc = next(
                                    i for i in range(len(KCW))
                                    if KCO[i] <= kb * P < KCO[i] + KCW[i]
                                )
                                ko = kb * P - KCO[kc]
                                nc.tensor.matmul(
                                    s_ps[:, h, :],
                                    lhsT=kt_ch[kc][:, ko:ko + P],
                                    rhs=qt_ch[qt][:],
                                    start=True,
                                    stop=True,
                                )
                            path = PATHS[t]
                            att = attnp.tile(
                                [P, 2, QW], bf16, name="att", tag="att"
                            )
                            if path == "SCHR":
                                pos = M8_POS[t]
                                att1 = attnep.tile(
                                    [P, 2, QW], bf16, name="att1", tag="atte"
                                )
                                att2 = attnep.tile(
                                    [P, 2, QW], bf16, name="att2", tag="atte"
                                )
                                # PSUM-reading ops stay on DVE (GPSIMD may
                                # not access PSUM); SBUF-only tail on Pool
                                nc.vector.scalar_tensor_tensor(
                                    out=att1[:].bitcast(i16),
                                    in0=m8_t[qt][:, 2 * pos:2 * pos + 2, :],
                                    scalar=mscale,
                                    in1=s_ps[:],
                                    op0=mybir.AluOpType.mult,
                                    op1=mybir.AluOpType.add,
                                )
                                nc.vector.tensor_scalar(
                                    out=att2[:].bitcast(i16),
                                    in0=att1[:].bitcast(i16),
                                    scalar1=64,
                                    scalar2=None,
                                    op0=mybir.AluOpType.add,
                                )
                                nc.vector.tensor_scalar(
                                    out=att2[:],
                                    in0=att2[:],
                                    scalar1=invs2,
                                    scalar2=None,
                                    op0=mybir.AluOpType.mult,
                                )
                                nc.gpsimd.tensor_add(
                                    att[:], att2[:], att1[:]
                                )
                            else:
                                att_e = attnep.tile(
                                    [P, 2, QW], bf16, name="att_e", tag="atte"
                                )
                                nc.scalar.activation(
                                    att_e[:],
                                    s_ps[:],
                                    mybir.ActivationFunctionType.Exp,
                                    scale=escale,
                                )
                                if path == "E16":
                                    pos = M16_POS[t]
                                    nc.vector.tensor_mul(
                                        att[:], att_e[:],
                                        m16_t[qt][:, 2 * pos:2 * pos + 2, :],
                                    )
                                else:  # EU8P
                                    pos = M8_POS[t]
                                    nc.gpsimd.tensor_mul(
                                        att[:], att_e[:],
                                        m8_t[qt][:, 2 * pos:2 * pos + 2, :],
                                    )
                            atts[t] = att
                    emit_norm((av_t, o_s, b, qt))
    nc.compile()
    return nc


def _prep_inputs(q, k, v, mask):
    bf = ml_dtypes.bfloat16
    ones = np.ones((B, S, 1), dtype=np.float32)
    va_full = (
        np.concatenate([np.asarray(v, np.float32), ones], axis=2)
        .reshape(B, S // P, P, D + 1)
        .transpose(0, 2, 1, 3)
        .reshape(B, P, (S // P) * (D + 1))
        .astype(bf)
    )
    ascale = np.float32(P / np.log(2.0) / np.sqrt(D))
    qT_full = np.ascontiguousarray(
        np.asarray(q, np.float32).transpose(0, 2, 1) * ascale
    ).astype(bf)
    kT_full = np.ascontiguousarray(
        np.asarray(k, np.float32).transpose(0, 2, 1)
    ).astype(bf)
    mT = np.asarray(mask, bool).transpose(0, 2, 1)  # [b, k, q]
    m16_full = np.empty((B, len(M16_PAIRS) * 2 * P, S), dtype=np.int16)
    for i, pr in enumerate(M16_PAIRS):
        blk = mT[:, 2 * pr * P:(2 * pr + 2) * P, :]
        r0, r1 = 2 * i * P, (2 * i + 2) * P
        m16_full[:, r0:r1, :] = np.where(blk, np.int16(0), np.int16(1))
    m8_full = np.empty((B, len(M8_PAIRS) * 2 * P, S), dtype=np.uint8)
    for i, pr in enumerate(M8_PAIRS):
        blk = mT[:, 2 * pr * P:(2 * pr + 2) * P, :]
        r0, r1 = 2 * i * P, (2 * i + 2) * P
        if PATHS[pr] == "SCHR":
            m8_full[:, r0:r1, :] = np.where(
                blk, np.uint8(SCHR_M_DEAD), np.uint8(SCHR_M_LIVE)
            )
        else:
            m8_full[:, r0:r1, :] = (~blk).astype(np.uint8)
    return qT_full, kT_full, va_full, m16_full, m8_full


def kernel(q, k, v, mask, _trace=False, _trace_kwargs=None):
    global _NC, LAST_RESULT
    from concourse.bass_utils import run_bass_kernel_spmd

    if _NC is None:
        _NC = _build_nc()

    qT_full, kT_full, va_full, m16_full, m8_full = _prep_inputs(q, k, v, mask)

    in_maps = []
    for c in range(N_CORES):
        lo, hi = c * BPC, (c + 1) * BPC
        in_maps.append(
            {
                "qT": qT_full[lo:hi],
                "kT": kT_full[lo:hi],
                "va": va_full[lo:hi],
                "m16": m16_full[lo:hi],
                "m8": m8_full[lo:hi],
            }
        )

    kw = {}
    if _trace:
        kw["trace"] = True
        if _trace_kwargs:
            kw.update(_trace_kwargs)
    LAST_RESULT = run_bass_kernel_spmd(_NC, in_maps, list(range(N_CORES)), **kw)
    res = LAST_RESULT.results
    outp = np.concatenate(
        [np.asarray(res[c]["out"]) for c in range(N_CORES)], axis=0
    ).astype(np.float32)
    # [b, qt, p, sl*D+d] -> [b, qt*512 + sl*128 + p, d]
    out = (
        outp.reshape(B, NQT, P, 4, D)
        .transpose(0, 1, 3, 2, 4)
        .reshape(B, S, D)
    )
    return np.ascontiguousarray(out)



# revision 3
# speedup vs baseline: 1.0723x; 1.0723x over previous
"""Masked attention on 8 TRN2 NeuronCores — pure data-parallel over batch.

Full inputs:  q,k,v (16,2048,128) f32, mask (16,2048,2048) bool.
Output:       (16,2048,128) f32.

Per core (2 batches). Scores are computed transposed S^T[k,q] = K·Q^T in bf16
on the PE so the AV contraction lands on the partition axis. The exp+mask
elementwise pass (8.4M elems/core) is split across engines by k-block pair
(SCHR pair-averaged Schraudolph on DVE/Pool, exact ACT exp + mask-mul on
DVE (E16) or Pool (EU8P)) exactly as before, but the schedule is a single
SOFTWARE-PIPELINED GLOBAL STREAM over all 64 (batch, q-tile, pair) steps:

  - Per step g: QK(g) matmuls, elementwise(g), then the AV matmuls of step
    g-PD. AV lags globally, so a pass's trailing AV blocks interleave with
    the NEXT pass's QK matmuls in the PE queue instead of serializing the
    pass boundary (the baseline lost ~3.1us/pass to this).
  - Mask DMAs are merged to one m8 + one m16 load per q-tile (the HWDGE
    descriptor-generator is a single ~625ns/DMA device shared by SP/ACT/DVE
    queues; at 94 DMAs it was a 59us serial resource, now ~48 DMAs).
  - All loads for both batches are issued up-front in deadline order on SP;
    the first q-tile's masks ride ACT/DVE queues so the head doesn't wait
    on SP's ~650ns/DMA issue rate. Stores stay on SP — they sit after all
    loads in SP's in-order queue, so their norm-waits can't block a load.
  - AV accumulates [q,129] per slot with a ones-column in va giving the
    softmax denominator; normalization is DVE reciprocal + scale, stored
    as [q, d] rows from SP's queue (host unpermutes).
"""

import numpy as np
import ml_dtypes

B, S, D = 16, 2048, 128
N_CORES = 8
BPC = B // N_CORES   # batches per core
P = 128              # partitions
QW = 512             # q-tile width
NQT = S // QW        # q-tiles per batch
KB = S // P          # k-blocks per batch
NPAIR = KB // 2      # k-block pairs per q-tile pass
PD = 5               # AV matmuls pipelined this many pairs behind elementwise
SPSUM_BUFS = 3       # scores double/triple buffering (2 banks each)
AV_BUFS = 2          # AV accumulator tiles (1 bank each)
ATT_BUFS = 8
ATTE_BUFS = 9
WARM_N = 14
M8_BUFS = 5
M16_BUFS = 5

# Per-pair elementwise path (8 entries, k-block pairs 0..7):
#   SCHR: pair-averaged Schraudolph, DVE stt + DVE/Pool tail (m8 bias codes)
#   E16:  ACT exp + DVE tensor_mul (m16 0/1 int16 rows)
#   EU8P: ACT exp + Pool stt-mul (m8 uint8 rows)
PATHS = ["SCHR", "SCHR", "E16", "EU8P", "EU8P", "EU8P", "EU8P", "E16"]
M16_PAIRS = [i for i, p in enumerate(PATHS) if p == "E16"]
M8_PAIRS = [i for i, p in enumerate(PATHS) if p in ("SCHR", "EU8P")]
M16_POS = {pr: i for i, pr in enumerate(M16_PAIRS)}
M8_POS = {pr: i for i, pr in enumerate(M8_PAIRS)}
SCHR_BIAS_LIVE = 16256.0 - 134.0
SCHR_M_LIVE = 252
SCHR_M_DEAD = 24
SCHR_MSCALE = SCHR_BIAS_LIVE / SCHR_M_LIVE  # 63.976...: m*this = bias

_NC = None
LAST_RESULT = None   # BassKernelResults of the most recent run (for profiling)


def _build_nc(bpc=BPC, s=S):
    import concourse.bacc as bacc
    import concourse.tile as tile
    from concourse import mybir

    bf16 = mybir.dt.bfloat16
    f32 = mybir.dt.float32
    i16 = mybir.dt.int16
    u8 = mybir.dt.uint8

    # qT is host-pre-scaled by a = 128/(ln2*sqrt(D)), so PSUM scores are
    # already in Schraudolph units; exact exp just rescales by ln2/128
    escale = float(np.log(2.0) / P)
    invs2 = float(1.0 / np.sqrt(2.0))
    mscale = float(SCHR_MSCALE)

    n16 = len(M16_PAIRS) * 2     # k-blocks in m16
    n8 = len(M8_PAIRS) * 2       # k-blocks in m8

    nc = bacc.Bacc()
    qT = nc.declare_dram_parameter("qT", [bpc, P, s], bf16, isOutput=False)
    kT = nc.declare_dram_parameter("kT", [bpc, P, s], bf16, isOutput=False)
    # va host-packed as [p, kb*(D+1)]: row p holds v[kb*128+p, :]+[1] per kb
    va = nc.declare_dram_parameter(
        "va", [bpc, P, KB * (D + 1)], bf16, isOutput=False
    )
    # m16[b, i*2P+p, q] int16, i indexes M16_PAIRS: 0/1 not-mask rows for
    # E16 pairs
    m16 = nc.declare_dram_parameter("m16", [bpc, n16 * P, s], i16, isOutput=False)
    # m8: uint8 rows for SCHR pairs (bias codes {252 live, 24 dead}) and
    # EU8P pairs ({1, 0} multiplier), indexed by M8_PAIRS position
    m8 = nc.declare_dram_parameter("m8", [bpc, n8 * P, s], u8, isOutput=False)
    # out packed [b, qt, p, sl*D+d] so each partition row is one contiguous
    # 1KB chunk (avoids the <512B DMA latency penalty); host unpermutes
    out = nc.declare_dram_parameter(
        "out", [bpc, NQT, P, 4 * D], bf16, isOutput=True
    )

    KCW = [2 * P, 2 * P, 4 * P, 4 * P, 4 * P]
    KCO = [0, 2 * P, 4 * P, 8 * P, 12 * P]

    with tile.TileContext(nc) as tc:
        with (
            tc.tile_pool(name="qk", bufs=2) as qkp,
            tc.tile_pool(name="vp", bufs=2) as vp,
            tc.tile_pool(name="m16p", bufs=M16_BUFS) as m16p,
            tc.tile_pool(name="m8p", bufs=M8_BUFS) as m8p,
            tc.tile_pool(name="attn", bufs=ATT_BUFS) as attnp,
            tc.tile_pool(name="attne", bufs=ATTE_BUFS) as attnep,
            tc.tile_pool(name="outp", bufs=3) as outp,
            tc.tile_pool(name="const", bufs=1) as constp,
            tc.tile_pool(name="rp", bufs=12) as rp,
            tc.tile_pool(name="spsum", bufs=SPSUM_BUFS, space="PSUM") as spsum,
            tc.tile_pool(name="avpsum", bufs=AV_BUFS, space="PSUM") as avpsum,
        ):
            # ACT warm-up: load the Exp table (~1.3us) during initial DMAs
            warm = constp.tile([P, 128], bf16, name="warm", tag="warm")
            nc.vector.memset(warm[:], 0.0)
            nc.scalar.activation(
                warm[:, 0:1], warm[:, 0:1], mybir.ActivationFunctionType.Exp
            )
            # PE warm-up burst: p-state ramps to full clock with sustained
            # busy; burn the initial DMA-wait window on dummy matmuls
            wp = avpsum.tile([P, 2, D + 1], f32, name="warm_mm", tag="av")
            for _ in range(WARM_N):
                nc.tensor.matmul(
                    wp[:, 0, 0:P], lhsT=warm[:], rhs=warm[:],
                    start=True, stop=True,
                )

            # ---- per-batch tiles + loads, deadline-ordered -------------
            kt_ch = [[None] * len(KCW) for _ in range(bpc)]
            qt_ch = [[None] * NQT for _ in range(bpc)]
            va_s = [None] * bpc
            m16_t = [[None] * NQT for _ in range(bpc)]
            m8_t = [[None] * NQT for _ in range(bpc)]

            for b in range(bpc):
                kt_ch[b] = [
                    qkp.tile([P, w], bf16, name="kt_ch", tag=f"kt{i}", bufs=2)
                    for i, w in enumerate(KCW)
                ]
                qt_ch[b] = [
                    qkp.tile([P, QW], bf16, name="qt_ch", tag="qt",
                             bufs=NQT + 2)
                    for _ in range(NQT)
                ]
                va_s[b] = vp.tile([P, KB, D + 1], bf16, name="va_s", tag="va")
                m16_t[b] = [
                    m16p.tile([P, n16, QW], i16, name="m16_t", tag="m16",
                              bufs=M16_BUFS)
                    for _ in range(NQT)
                ]
                m8_t[b] = [
                    m8p.tile([P, n8, QW], u8, name="m8_t", tag="m8",
                             bufs=M8_BUFS)
                    for _ in range(NQT)
                ]

            def _ldk(b, i, eng):
                eng.dma_start(
                    out=kt_ch[b][i][:], in_=kT[b, :, KCO[i]:KCO[i] + KCW[i]]
                )

            def _ldq(b, qt, eng):
                eng.dma_start(
                    out=qt_ch[b][qt][:], in_=qT[b, :, qt * QW:(qt + 1) * QW]
                )

            def _ldva(b, lo, hi, eng):
                vr = va[b, :, :].rearrange("p (kb d) -> p kb d", d=D + 1)
                eng.dma_start(out=va_s[b][:, lo:hi, :], in_=vr[:, lo:hi, :])

            def _ldm16(b, qt, eng):
                m16r = m16[b, :, :].rearrange("(kb p) q -> p kb q", p=P)
                eng.dma_start(
                    out=m16_t[b][qt][:],
                    in_=m16r[:, :, qt * QW:(qt + 1) * QW],
                )

            def _ldm8(b, qt, eng, lo=0, hi=None):
                hi = n8 if hi is None else hi
                m8r = m8[b, :, :].rearrange("(kb p) q -> p kb q", p=P)
                eng.dma_start(
                    out=m8_t[b][qt][:, lo:hi, :],
                    in_=m8r[:, lo:hi, qt * QW:(qt + 1) * QW],
                )

            SP, ACT, POOL = nc.sync, nc.scalar, nc.gpsimd
            # batch 0, q-tile 0: masks split across ACT/Pool queues so the
            # first pass isn't gated on SP's ~650ns/DMA issue rate
            _ldk(0, 0, SP)
            _ldq(0, 0, SP)
            _ldm8(0, 0, SP, 0, 4)        # SCHR pairs 0,1
            _ldm16(0, 0, ACT)            # E16 pairs 2,7
            _ldm8(0, 0, ACT, 4, 8)       # EU8P pairs 3,4
            _ldm8(0, 0, ACT, 8, 12)      # EU8P pairs 5,6
            _ldva(0, 0, 8, POOL)
            _ldva(0, 8, KB, POOL)
            _ldk(0, 1, SP)
            _ldk(0, 2, SP)
            _ldk(0, 3, SP)
            _ldk(0, 4, SP)
            for qt in range(1, NQT):
                _ldq(0, qt, SP)
                _ldm8(0, qt, SP)
                _ldm16(0, qt, SP)
            # batch 1: all on SP, deadline order
            _ldk(1, 0, SP)
            _ldq(1, 0, SP)
            _ldva(1, 0, 8, SP)
            _ldm8(1, 0, SP)
            _ldm16(1, 0, SP)
            _ldk(1, 1, SP)
            _ldva(1, 8, KB, SP)
            _ldk(1, 2, SP)
            _ldk(1, 3, SP)
            _ldk(1, 4, SP)
            for qt in range(1, NQT):
                _ldq(1, qt, SP)
                _ldm8(1, qt, SP)
                _ldm16(1, qt, SP)

            # ---- merged global pair stream -----------------------------
            def emit_norm(av_t, o_s, b_, qt_):
                for sl in range(4):
                    avap = av_t[sl // 2][:, sl % 2, :]
                    recip = rp.tile([P, 1], f32, name="recip", tag="recip")
                    nc.vector.reciprocal(recip[:], avap[:, D:D + 1])
                    nc.vector.tensor_scalar_mul(
                        o_s[:, sl, :], avap[:, 0:D], recip[:]
                    )
                nc.sync.dma_start(
                    out=out[b_, qt_, :, :].rearrange(
                        "p (sl d) -> p sl d", d=D
                    ),
                    in_=o_s[:, :, :],
                )

            TOT = bpc * NQT * NPAIR
            atts = [None] * TOT
            avs = {}    # pass index -> (av_t list, o_s)

            for g in range(TOT + PD):
                if g < TOT:
                    b = g // (NQT * NPAIR)
                    qt = (g // NPAIR) % NQT
                    t = g % NPAIR
                    s_ps = spsum.tile([P, 2, QW], f32, name="s_ps", tag="s")
                    for h in range(2):
                        kb = 2 * t + h
                        kc = next(
                            i for i in range(len(KCW))
                            if KCO[i] <= kb * P < KCO[i] + KCW[i]
                        )
                        ko = kb * P - KCO[kc]
                        nc.tensor.matmul(
                            s_ps[:, h, :],
                            lhsT=kt_ch[b][kc][:, ko:ko + P],
                            rhs=qt_ch[b][qt][:],
                            start=True,
                            stop=True,
                        )
                    path = PATHS[t]
                    att = attnp.tile([P, 2, QW], bf16, name="att", tag="att")
                    if path == "SCHR":
                        pos = M8_POS[t]
                        att1 = attnep.tile(
                            [P, 2, QW], bf16, name="att1", tag="atte"
                        )
                        att2 = attnep.tile(
                            [P, 2, QW], bf16, name="att2", tag="atte"
                        )
                        # PSUM-reading ops stay on DVE (GPSIMD may not
                        # access PSUM); SBUF-only tail split DVE/Pool
                        nc.vector.scalar_tensor_tensor(
                            out=att1[:].bitcast(i16),
                            in0=m8_t[b][qt][:, 2 * pos:2 * pos + 2, :],
                            scalar=mscale,
                            in1=s_ps[:],
                            op0=mybir.AluOpType.mult,
                            op1=mybir.AluOpType.add,
                        )
                        nc.vector.tensor_scalar(
                            out=att2[:].bitcast(i16),
                            in0=att1[:].bitcast(i16),
                            scalar1=64,
                            scalar2=None,
                            op0=mybir.AluOpType.add,
                        )
                        nc.vector.tensor_scalar(
                            out=att2[:],
                            in0=att2[:],
                            scalar1=invs2,
                            scalar2=None,
                            op0=mybir.AluOpType.mult,
                        )
                        nc.gpsimd.tensor_add(att[:], att2[:], att1[:])
                    else:
                        att_e = attnep.tile(
                            [P, 2, QW], bf16, name="att_e", tag="atte"
                        )
                        nc.scalar.activation(
                            att_e[:],
                            s_ps[:],
                            mybir.ActivationFunctionType.Exp,
                            scale=escale,
                        )
                        if path == "E16":
                            pos = M16_POS[t]
                            nc.vector.tensor_mul(
                                att[:], att_e[:],
                                m16_t[b][qt][:, 2 * pos:2 * pos + 2, :],
                            )
                        else:  # EU8P
                            pos = M8_POS[t]
                            nc.gpsimd.tensor_mul(
                                att[:], att_e[:],
                                m8_t[b][qt][:, 2 * pos:2 * pos + 2, :],
                            )
                    atts[g] = att

                gp = g - PD
                if gp >= 0:
                    bp = gp // (NQT * NPAIR)
                    tp = gp % NPAIR
                    pidx = gp // NPAIR
                    if tp == 0:
                        avs[pidx] = (
                            [
                                avpsum.tile(
                                    [P, 2, D + 1], f32, name="av", tag="av"
                                )
                                for _ in range(2)
                            ],
                            outp.tile([P, 4, D], bf16, name="o_s", tag="o_s"),
                        )
                    av_t, o_s = avs[pidx]
                    for h in range(2):
                        kb = 2 * tp + h
                        for sl in range(4):
                            nc.tensor.matmul(
                                av_t[sl // 2][:, sl % 2, :],
                                lhsT=atts[gp][:, h, sl * P:(sl + 1) * P],
                                rhs=va_s[bp][:, kb, :],
                                start=(kb == 0 and sl % 2 == 0),
                                stop=(kb == KB - 1 and sl % 2 == 1),
                            )
                    if tp == NPAIR - 1:
                        qtp = (gp // NPAIR) % NQT
                        emit_norm(av_t, o_s, bp, qtp)
                        del avs[pidx]
    nc.compile()
    return nc


def _prep_inputs(q, k, v, mask):
    bf = ml_dtypes.bfloat16
    ones = np.ones((B, S, 1), dtype=np.float32)
    va_full = (
        np.concatenate([np.asarray(v, np.float32), ones], axis=2)
        .reshape(B, S // P, P, D + 1)
        .transpose(0, 2, 1, 3)
        .reshape(B, P, (S // P) * (D + 1))
        .astype(bf)
    )
    ascale = np.float32(P / np.log(2.0) / np.sqrt(D))
    qT_full = np.ascontiguousarray(
        np.asarray(q, np.float32).transpose(0, 2, 1) * ascale
    ).astype(bf)
    kT_full = np.ascontiguousarray(
        np.asarray(k, np.float32).transpose(0, 2, 1)
    ).astype(bf)
    mT = np.asarray(mask, bool).transpose(0, 2, 1)  # [b, k, q]
    m16_full = np.empty((B, len(M16_PAIRS) * 2 * P, S), dtype=np.int16)
    for i, pr in enumerate(M16_PAIRS):
        blk = mT[:, 2 * pr * P:(2 * pr + 2) * P, :]
        r0, r1 = 2 * i * P, (2 * i + 2) * P
        m16_full[:, r0:r1, :] = np.where(blk, np.int16(0), np.int16(1))
    m8_full = np.empty((B, len(M8_PAIRS) * 2 * P, S), dtype=np.uint8)
    for i, pr in enumerate(M8_PAIRS):
        blk = mT[:, 2 * pr * P:(2 * pr + 2) * P, :]
        r0, r1 = 2 * i * P, (2 * i + 2) * P
        if PATHS[pr] == "SCHR":
            m8_full[:, r0:r1, :] = np.where(
                blk, np.uint8(SCHR_M_DEAD), np.uint8(SCHR_M_LIVE)
            )
        else:
            m8_full[:, r0:r1, :] = (~blk).astype(np.uint8)
    return qT_full, kT_full, va_full, m16_full, m8_full


def kernel(q, k, v, mask, _trace=False, _trace_kwargs=None):
    global _NC, LAST_RESULT
    from concourse.bass_utils import run_bass_kernel_spmd

    if _NC is None:
        _NC = _build_nc()

    qT_full, kT_full, va_full, m16_full, m8_full = _prep_inputs(q, k, v, mask)

    in_maps = []
    for c in range(N_CORES):
        lo, hi = c * BPC, (c + 1) * BPC
        in_maps.append(
            {
                "qT": qT_full[lo:hi],
                "kT": kT_full[lo:hi],
                "va": va_full[lo:hi],
                "m16": m16_full[lo:hi],
                "m8": m8_full[lo:hi],
            }
        )

    kw = {}
    if _trace:
        kw["trace"] = True
        if _trace_kwargs:
            kw.update(_trace_kwargs)
    LAST_RESULT = run_bass_kernel_spmd(_NC, in_maps, list(range(N_CORES)), **kw)
    res = LAST_RESULT.results
    outp = np.concatenate(
        [np.asarray(res[c]["out"]) for c in range(N_CORES)], axis=0
    ).astype(np.float32)
    # [b, qt, p, sl*D+d] -> [b, qt*512 + sl*128 + p, d]
    out = (
        outp.reshape(B, NQT, P, 4, D)
        .transpose(0, 1, 3, 2, 4)
        .reshape(B, S, D)
    )
    return np.ascontiguousarray(out)


# revision 8
# speedup vs baseline: 1.1046x; 1.0302x over previous
"""Masked attention on 8 TRN2 NeuronCores — pure data-parallel over batch.

Full inputs:  q,k,v (16,2048,128) f32, mask (16,2048,2048) bool.
Output:       (16,2048,128) f32.

Per core (2 batches). Scores are computed transposed S^T[k,q] = K·Q^T in bf16
on the PE so the AV contraction lands on the partition axis. The exp+mask
elementwise pass (8.4M elems/core) is split across engines by k-block pair
(SCHR pair-averaged Schraudolph on DVE/Pool, exact ACT exp + mask-mul on
DVE (E16) or Pool (EU8P)) exactly as before, but the schedule is a single
SOFTWARE-PIPELINED GLOBAL STREAM over all 64 (batch, q-tile, pair) steps:

  - Per step g: QK(g) matmuls, elementwise(g), then the AV matmuls of step
    g-PD. AV lags globally, so a pass's trailing AV blocks interleave with
    the NEXT pass's QK matmuls in the PE queue instead of serializing the
    pass boundary (the baseline lost ~3.1us/pass to this).
  - Mask DMAs are merged to one m8 + one m16 load per q-tile (the HWDGE
    descriptor-generator is a single ~625ns/DMA device shared by SP/ACT/DVE
    queues; at 94 DMAs it was a 59us serial resource, now ~48 DMAs).
  - All loads for both batches are issued up-front in deadline order on SP;
    the first q-tile's masks ride ACT/DVE queues so the head doesn't wait
    on SP's ~650ns/DMA issue rate. Stores stay on SP — they sit after all
    loads in SP's in-order queue, so their norm-waits can't block a load.
  - AV accumulates [q,129] per slot with a ones-column in va giving the
    softmax denominator; normalization is DVE reciprocal + scale, stored
    as [q, d] rows from SP's queue (host unpermutes).
"""

import numpy as np
import ml_dtypes

B, S, D = 16, 2048, 128
N_CORES = 8
BPC = B // N_CORES   # batches per core
P = 128              # partitions
QW = 512             # q-tile width
NQT = S // QW        # q-tiles per batch
KB = S // P          # k-blocks per batch
NPAIR = KB // 2      # k-block pairs per q-tile pass
PD = 5               # AV matmuls pipelined this many pairs behind elementwise
SPSUM_BUFS = 3       # scores double/triple buffering (2 banks each)
AV_BUFS = 2          # AV accumulator tiles (1 bank each)
ATT_BUFS = 8
ATTE_BUFS = 8     # att_e (exp outputs); separate pool from SCHR intermediates
SCHRI_BUFS = 6    # att1/att2 SCHR intermediates
WARM_N = 14
M8_BUFS = 5
M16_BUFS = 5

# Per-pair elementwise path (8 entries, k-block pairs 0..7):
#   SCHR: pair-averaged Schraudolph, DVE stt + DVE/Pool tail (m8 bias codes)
#   E16:  ACT exp + DVE tensor_mul (m16 0/1 int16 rows)
#   EU8P: ACT exp + Pool stt-mul (m8 uint8 rows)
# Order within a pass: shortest-latency paths (E16) at the pair positions
# with the earliest/latest AV deadlines; the long SCHR chains sit mid-pass
# where their AV deadline (pair index + PD) gives them the most slack.
PATHS = ["E16", "EU8P", "SCHR", "EU8P", "SCHR", "EU8P", "EU8P", "E16"]
M16_PAIRS = [i for i, p in enumerate(PATHS) if p == "E16"]
M8_PAIRS = [i for i, p in enumerate(PATHS) if p in ("SCHR", "EU8P")]
M16_POS = {pr: i for i, pr in enumerate(M16_PAIRS)}
M8_POS = {pr: i for i, pr in enumerate(M8_PAIRS)}
SCHR_BIAS_LIVE = 16256.0 - 134.0
SCHR_M_LIVE = 252
SCHR_M_DEAD = 24
SCHR_MSCALE = SCHR_BIAS_LIVE / SCHR_M_LIVE  # 63.976...: m*this = bias

_NC = None
LAST_RESULT = None   # BassKernelResults of the most recent run (for profiling)


def _build_nc(bpc=BPC, s=S):
    import concourse.bacc as bacc
    import concourse.tile as tile
    from concourse import mybir

    bf16 = mybir.dt.bfloat16
    f32 = mybir.dt.float32
    i16 = mybir.dt.int16
    u8 = mybir.dt.uint8

    # qT is host-pre-scaled by a = 128/(ln2*sqrt(D)), so PSUM scores are
    # already in Schraudolph units; exact exp just rescales by ln2/128
    escale = float(np.log(2.0) / P)
    invs2 = float(1.0 / np.sqrt(2.0))
    mscale = float(SCHR_MSCALE)

    n16 = len(M16_PAIRS) * 2     # k-blocks in m16
    n8 = len(M8_PAIRS) * 2       # k-blocks in m8

    nc = bacc.Bacc()
    qT = nc.declare_dram_parameter("qT", [bpc, P, s], bf16, isOutput=False)
    kT = nc.declare_dram_parameter("kT", [bpc, P, s], bf16, isOutput=False)
    # va host-packed as [p, kb*(D+1)]: row p holds v[kb*128+p, :]+[1] per kb
    va = nc.declare_dram_parameter(
        "va", [bpc, P, KB * (D + 1)], bf16, isOutput=False
    )
    # m16[b, i*2P+p, q] int16, i indexes M16_PAIRS: 0/1 not-mask rows for
    # E16 pairs
    m16 = nc.declare_dram_parameter("m16", [bpc, n16 * P, s], i16, isOutput=False)
    # m8: uint8 rows for SCHR pairs (bias codes {252 live, 24 dead}) and
    # EU8P pairs ({1, 0} multiplier), indexed by M8_PAIRS position
    m8 = nc.declare_dram_parameter("m8", [bpc, n8 * P, s], u8, isOutput=False)
    # out packed [b, qt, p, sl*D+d] so each partition row is one contiguous
    # 1KB chunk (avoids the <512B DMA latency penalty); host unpermutes
    out = nc.declare_dram_parameter(
        "out", [bpc, NQT, P, 4 * D], bf16, isOutput=True
    )

    KCW = [2 * P, 2 * P, 4 * P, 4 * P, 4 * P]
    KCO = [0, 2 * P, 4 * P, 8 * P, 12 * P]

    with tile.TileContext(nc) as tc:
        with (
            tc.tile_pool(name="qk", bufs=2) as qkp,
            tc.tile_pool(name="vp", bufs=2) as vp,
            tc.tile_pool(name="m16p", bufs=M16_BUFS) as m16p,
            tc.tile_pool(name="m8p", bufs=M8_BUFS) as m8p,
            tc.tile_pool(name="attn", bufs=ATT_BUFS) as attnp,
            tc.tile_pool(name="attne", bufs=ATTE_BUFS) as attnep,
            tc.tile_pool(name="schri", bufs=SCHRI_BUFS) as schrip,
            tc.tile_pool(name="outp", bufs=3) as outp,
            tc.tile_pool(name="const", bufs=1) as constp,
            tc.tile_pool(name="rp", bufs=12) as rp,
            tc.tile_pool(name="spsum", bufs=SPSUM_BUFS, space="PSUM") as spsum,
            tc.tile_pool(name="avpsum", bufs=AV_BUFS, space="PSUM") as avpsum,
        ):
            # ACT warm-up: load the Exp table (~1.3us) during initial DMAs
            warm = constp.tile([P, 128], bf16, name="warm", tag="warm")
            nc.vector.memset(warm[:], 0.0)
            nc.scalar.activation(
                warm[:, 0:1], warm[:, 0:1], mybir.ActivationFunctionType.Exp
            )
            # PE warm-up burst: p-state ramps to full clock with sustained
            # busy; burn the initial DMA-wait window on dummy matmuls
            wp = avpsum.tile([P, 2, D + 1], f32, name="warm_mm", tag="av")
            for _ in range(WARM_N):
                nc.tensor.matmul(
                    wp[:, 0, 0:P], lhsT=warm[:], rhs=warm[:],
                    start=True, stop=True,
                )

            # ---- per-batch tiles + loads, deadline-ordered -------------
            kt_ch = [[None] * len(KCW) for _ in range(bpc)]
            qt_ch = [[None] * NQT for _ in range(bpc)]
            va_s = [None] * bpc
            m16_t = [[None] * NQT for _ in range(bpc)]
            m8_t = [[None] * NQT for _ in range(bpc)]

            for b in range(bpc):
                kt_ch[b] = [
                    qkp.tile([P, w], bf16, name="kt_ch", tag=f"kt{i}", bufs=2)
                    for i, w in enumerate(KCW)
                ]
                qt_ch[b] = [
                    qkp.tile([P, QW], bf16, name="qt_ch", tag="qt",
                             bufs=NQT + 2)
                    for _ in range(NQT)
                ]
                va_s[b] = vp.tile([P, KB, D + 1], bf16, name="va_s", tag="va")
                m16_t[b] = [
                    m16p.tile([P, n16, QW], i16, name="m16_t", tag="m16",
                              bufs=M16_BUFS)
                    for _ in range(NQT)
                ]
                m8_t[b] = [
                    m8p.tile([P, n8, QW], u8, name="m8_t", tag="m8",
                             bufs=M8_BUFS)
                    for _ in range(NQT)
                ]

            def _ldk(b, i, eng):
                eng.dma_start(
                    out=kt_ch[b][i][:], in_=kT[b, :, KCO[i]:KCO[i] + KCW[i]]
                )

            def _ldq(b, qt, eng):
                eng.dma_start(
                    out=qt_ch[b][qt][:], in_=qT[b, :, qt * QW:(qt + 1) * QW]
                )

            def _ldva(b, lo, hi, eng):
                vr = va[b, :, :].rearrange("p (kb d) -> p kb d", d=D + 1)
                eng.dma_start(out=va_s[b][:, lo:hi, :], in_=vr[:, lo:hi, :])

            def _ldm16(b, qt, eng):
                m16r = m16[b, :, :].rearrange("(kb p) q -> p kb q", p=P)
                eng.dma_start(
                    out=m16_t[b][qt][:],
                    in_=m16r[:, :, qt * QW:(qt + 1) * QW],
                )

            def _ldm8(b, qt, eng, lo=0, hi=None):
                hi = n8 if hi is None else hi
                m8r = m8[b, :, :].rearrange("(kb p) q -> p kb q", p=P)
                eng.dma_start(
                    out=m8_t[b][qt][:, lo:hi, :],
                    in_=m8r[:, lo:hi, qt * QW:(qt + 1) * QW],
                )

            SP, ACT, POOL = nc.sync, nc.scalar, nc.gpsimd
            # batch 0, q-tile 0: masks split across ACT/Pool queues so the
            # first pass isn't gated on SP's ~650ns/DMA issue rate
            _ldk(0, 0, SP)
            _ldq(0, 0, SP)
            _ldm16(0, 0, SP)             # E16 pairs 0,7 (pair 0 first)
            _ldm8(0, 0, ACT, 0, 4)       # pairs 1,2
            _ldm8(0, 0, ACT, 4, 8)       # pairs 3,4
            _ldm8(0, 0, ACT, 8, 12)     # pairs 5,6
            _ldva(0, 0, 8, POOL)
            _ldva(0, 8, KB, POOL)
            _ldk(0, 1, SP)
            _ldk(0, 2, SP)
            _ldk(0, 3, SP)
            _ldk(0, 4, SP)
            for qt in range(1, NQT):
                _ldq(0, qt, SP)
                _ldm8(0, qt, SP)
                _ldm16(0, qt, SP)
            # batch 1: all on SP, deadline order
            _ldk(1, 0, SP)
            _ldq(1, 0, SP)
            _ldva(1, 0, 8, SP)
            _ldm8(1, 0, SP)
            _ldm16(1, 0, SP)
            _ldk(1, 1, SP)
            _ldva(1, 8, KB, SP)
            _ldk(1, 2, SP)
            _ldk(1, 3, SP)
            _ldk(1, 4, SP)
            for qt in range(1, NQT):
                _ldq(1, qt, SP)
                _ldm8(1, qt, SP)
                _ldm16(1, qt, SP)

            # ---- merged global pair stream -----------------------------
            def emit_norm(av_t, o_s, b_, qt_):
                for sl in range(4):
                    avap = av_t[sl // 2][:, sl % 2, :]
                    recip = rp.tile([P, 1], f32, name="recip", tag="recip")
                    nc.vector.reciprocal(recip[:], avap[:, D:D + 1])
                    nc.vector.tensor_scalar_mul(
                        o_s[:, sl, :], avap[:, 0:D], recip[:]
                    )
                nc.sync.dma_start(
                    out=out[b_, qt_, :, :].rearrange(
                        "p (sl d) -> p sl d", d=D
                    ),
                    in_=o_s[:, :, :],
                )

            TOT = bpc * NQT * NPAIR
            atts = [None] * TOT
            avs = {}    # pass index -> (av_t list, o_s)

            for g in range(TOT + PD):
                if g < TOT:
                    b = g // (NQT * NPAIR)
                    qt = (g // NPAIR) % NQT
                    t = g % NPAIR
                    s_ps = spsum.tile([P, 2, QW], f32, name="s_ps", tag="s")
                    for h in range(2):
                        kb = 2 * t + h
                        kc = next(
                            i for i in range(len(KCW))
                            if KCO[i] <= kb * P < KCO[i] + KCW[i]
                        )
                        ko = kb * P - KCO[kc]
                        nc.tensor.matmul(
                            s_ps[:, h, :],
                            lhsT=kt_ch[b][kc][:, ko:ko + P],
                            rhs=qt_ch[b][qt][:],
                            start=True,
                            stop=True,
                        )
                    path = PATHS[t]
                    att = attnp.tile([P, 2, QW], bf16, name="att", tag="att")
                    if path == "SCHR":
                        pos = M8_POS[t]
                        att1 = schrip.tile(
                            [P, 2, QW], bf16, name="att1", tag="schri"
                        )
                        att2 = schrip.tile(
                            [P, 2, QW], bf16, name="att2", tag="schri"
                        )
                        # PSUM-reading ops stay on DVE (GPSIMD may not
                        # access PSUM); SBUF-only tail split DVE/Pool
                        nc.vector.scalar_tensor_tensor(
                            out=att1[:].bitcast(i16),
                            in0=m8_t[b][qt][:, 2 * pos:2 * pos + 2, :],
                            scalar=mscale,
                            in1=s_ps[:],
                            op0=mybir.AluOpType.mult,
                            op1=mybir.AluOpType.add,
                        )
                        nc.vector.tensor_scalar(
                            out=att2[:].bitcast(i16),
                            in0=att1[:].bitcast(i16),
                            scalar1=64,
                            scalar2=None,
                            op0=mybir.AluOpType.add,
                        )
                        nc.vector.tensor_scalar(
                            out=att2[:],
                            in0=att2[:],
                            scalar1=invs2,
                            scalar2=None,
                            op0=mybir.AluOpType.mult,
                        )
                        nc.gpsimd.tensor_add(att[:], att2[:], att1[:])
                    else:
                        att_e = attnep.tile(
                            [P, 2, QW], bf16, name="att_e", tag="atte"
                        )
                        nc.scalar.activation(
                            att_e[:],
                            s_ps[:],
                            mybir.ActivationFunctionType.Exp,
                            scale=escale,
                        )
                        if path == "E16":
                            pos = M16_POS[t]
                            nc.vector.tensor_mul(
                                att[:], att_e[:],
                                m16_t[b][qt][:, 2 * pos:2 * pos + 2, :],
                            )
                        else:  # EU8P
                            pos = M8_POS[t]
                            nc.gpsimd.tensor_mul(
                                att[:], att_e[:],
                                m8_t[b][qt][:, 2 * pos:2 * pos + 2, :],
                            )
                    atts[g] = att

                gp = g - PD
                if gp >= 0:
                    bp = gp // (NQT * NPAIR)
                    tp = gp % NPAIR
                    pidx = gp // NPAIR
                    if tp == 0:
                        avs[pidx] = (
                            [
                                avpsum.tile(
                                    [P, 2, D + 1], f32, name="av", tag="av"
                                )
                                for _ in range(2)
                            ],
                            outp.tile([P, 4, D], bf16, name="o_s", tag="o_s"),
                        )
                    av_t, o_s = avs[pidx]
                    for h in range(2):
                        kb = 2 * tp + h
                        for sl in range(4):
                            nc.tensor.matmul(
                                av_t[sl // 2][:, sl % 2, :],
                                lhsT=atts[gp][:, h, sl * P:(sl + 1) * P],
                                rhs=va_s[bp][:, kb, :],
                                start=(kb == 0 and sl % 2 == 0),
                                stop=(kb == KB - 1 and sl % 2 == 1),
                            )
                    if tp == NPAIR - 1:
                        qtp = (gp // NPAIR) % NQT
                        emit_norm(av_t, o_s, bp, qtp)
                        del avs[pidx]
    nc.compile()
    return nc


def _prep_inputs(q, k, v, mask):
    bf = ml_dtypes.bfloat16
    ones = np.ones((B, S, 1), dtype=np.float32)
    va_full = (
        np.concatenate([np.asarray(v, np.float32), ones], axis=2)
        .reshape(B, S // P, P, D + 1)
        .transpose(0, 2, 1, 3)
        .reshape(B, P, (S // P) * (D + 1))
        .astype(bf)
    )
    ascale = np.float32(P / np.log(2.0) / np.sqrt(D))
    qT_full = np.ascontiguousarray(
        np.asarray(q, np.float32).transpose(0, 2, 1) * ascale
    ).astype(bf)
    kT_full = np.ascontiguousarray(
        np.asarray(k, np.float32).transpose(0, 2, 1)
    ).astype(bf)
    mT = np.asarray(mask, bool).transpose(0, 2, 1)  # [b, k, q]
    m16_full = np.empty((B, len(M16_PAIRS) * 2 * P, S), dtype=np.int16)
    for i, pr in enumerate(M16_PAIRS):
        blk = mT[:, 2 * pr * P:(2 * pr + 2) * P, :]
        r0, r1 = 2 * i * P, (2 * i + 2) * P
        m16_full[:, r0:r1, :] = np.where(blk, np.int16(0), np.int16(1))
    m8_full = np.empty((B, len(M8_PAIRS) * 2 * P, S), dtype=np.uint8)
    for i, pr in enumerate(M8_PAIRS):
        blk = mT[:, 2 * pr * P:(2 * pr + 2) * P, :]
        r0, r1 = 2 * i * P, (2 * i + 2) * P
        if PATHS[pr] == "SCHR":
            m8_full[:, r0:r1, :] = np.where(
                blk, np.uint8(SCHR_M_DEAD), np.uint8(SCHR_M_LIVE)
            )
        else:
            m8_full[:, r0:r1, :] = (~blk).astype(np.uint8)
    return qT_full, kT_full, va_full, m16_full, m8_full


def kernel(q, k, v, mask, _trace=False, _trace_kwargs=None):
    global _NC, LAST_RESULT
    from concourse.bass_utils import run_bass_kernel_spmd

    if _NC is None:
        _NC = _build_nc()

    qT_full, kT_full, va_full, m16_full, m8_full = _prep_inputs(q, k, v, mask)

    in_maps = []
    for c in range(N_CORES):
        lo, hi = c * BPC, (c + 1) * BPC
        in_maps.append(
            {
                "qT": qT_full[lo:hi],
                "kT": kT_full[lo:hi],
                "va": va_full[lo:hi],
                "m16": m16_full[lo:hi],
                "m8": m8_full[lo:hi],
            }
        )

    kw = {}
    if _trace:
        kw["trace"] = True
        if _trace_kwargs:
            kw.update(_trace_kwargs)
    LAST_RESULT = run_bass_kernel_spmd(_NC, in_maps, list(range(N_CORES)), **kw)
    res = LAST_RESULT.results
    outp = np.concatenate(
        [np.asarray(res[c]["out"]) for c in range(N_CORES)], axis=0
    ).astype(np.float32)
    # [b, qt, p, sl*D+d] -> [b, qt*512 + sl*128 + p, d]
    out = (
        outp.reshape(B, NQT, P, 4, D)
        .transpose(0, 1, 3, 2, 4)
        .reshape(B, S, D)
    )
    return np.ascontiguousarray(out)


# revision 13
# speedup vs baseline: 1.1812x; 1.0693x over previous
"""Masked attention on 8 TRN2 NeuronCores — pure data-parallel over batch.

Full inputs:  q,k,v (16,2048,128) f32, mask (16,2048,2048) bool.
Output:       (16,2048,128) f32.

Per core (2 batches). Scores are computed transposed S^T[k,q] = K·Q^T in bf16
on the PE so the AV contraction lands on the partition axis. The exp+mask
elementwise pass (8.4M elems/core) is split across engines by k-block pair
(SCHR pair-averaged Schraudolph on DVE/Pool, exact ACT exp + mask-mul on
DVE (E16) or Pool (EU8P)) exactly as before, but the schedule is a single
SOFTWARE-PIPELINED GLOBAL STREAM over all 64 (batch, q-tile, pair) steps:

  - Per step g: QK(g) matmuls, elementwise(g), then the AV matmuls of step
    g-PD. AV lags globally, so a pass's trailing AV blocks interleave with
    the NEXT pass's QK matmuls in the PE queue instead of serializing the
    pass boundary (the baseline lost ~3.1us/pass to this).
  - Mask DMAs are merged to one m8 + one m16 load per q-tile (the HWDGE
    descriptor-generator is a single ~625ns/DMA device shared by SP/ACT/DVE
    queues; at 94 DMAs it was a 59us serial resource, now ~48 DMAs).
  - All loads for both batches are issued up-front in deadline order on SP;
    the first q-tile's masks ride ACT/DVE queues so the head doesn't wait
    on SP's ~650ns/DMA issue rate. Stores stay on SP — they sit after all
    loads in SP's in-order queue, so their norm-waits can't block a load.
  - AV accumulates [q,129] per slot with a ones-column in va giving the
    softmax denominator; normalization is DVE reciprocal + scale, stored
    as [q, d] rows from SP's queue (host unpermutes).
"""

import numpy as np
import ml_dtypes

B, S, D = 16, 2048, 128
N_CORES = 8
BPC = B // N_CORES   # batches per core
P = 128              # partitions
QW = 512             # q-tile width
NQT = S // QW        # q-tiles per batch
KB = S // P          # k-blocks per batch
NPAIR = KB // 2      # k-block pairs per q-tile pass
PD = 6               # AV matmuls pipelined this many pairs behind elementwise
SPSUM_BUFS = 3       # scores double/triple buffering (2 banks each)
AV_BUFS = 2          # AV accumulator tiles (1 bank each)
ATT_BUFS = 9
ATTE_BUFS = 8     # att_e (exp outputs); separate pool from SCHR intermediates
SCHRI_BUFS = 6    # att1/att2 SCHR intermediates
WARM_N = 14
M8_BUFS = 5
M16_BUFS = 5

# Per-pair elementwise path (8 entries, k-block pairs 0..7):
#   SCHR: pair-averaged Schraudolph, DVE stt + DVE/Pool tail (m8 bias codes)
#   E16:  ACT exp + DVE tensor_mul (m16 0/1 int16 rows)
#   EU8P: ACT exp + Pool stt-mul (m8 uint8 rows)
# Order within a pass: shortest-latency paths (E16) at the pair positions
# with the earliest/latest AV deadlines; the long SCHR chains sit at {2,6}
# so ACT's six serialized exps never run more than 3 back-to-back (exp
# completion spacing 1.038us vs QK spacing 0.857us — longer runs make the
# s_ps WAR bite), and their AV deadline (pair index + PD) has slack.
PATHS = ["E16", "EU8P", "SCHR", "EU8P", "EU8P", "EU8P", "SCHR", "E16"]
M16_PAIRS = [i for i, p in enumerate(PATHS) if p == "E16"]
M8_PAIRS = [i for i, p in enumerate(PATHS) if p in ("SCHR", "EU8P")]
M16_POS = {pr: i for i, pr in enumerate(M16_PAIRS)}
M8_POS = {pr: i for i, pr in enumerate(M8_PAIRS)}
SCHR_BIAS_LIVE = 16256.0 - 134.0
SCHR_M_LIVE = 252
SCHR_M_DEAD = 24
SCHR_MSCALE = SCHR_BIAS_LIVE / SCHR_M_LIVE  # 63.976...: m*this = bias

_NC = None
LAST_RESULT = None   # BassKernelResults of the most recent run (for profiling)


def _build_nc(bpc=BPC, s=S):
    import concourse.bacc as bacc
    import concourse.tile as tile
    from concourse import mybir

    bf16 = mybir.dt.bfloat16
    f32 = mybir.dt.float32
    i16 = mybir.dt.int16
    u8 = mybir.dt.uint8

    # qT is host-pre-scaled by a = 128/(ln2*sqrt(D)), so PSUM scores are
    # already in Schraudolph units; exact exp just rescales by ln2/128
    escale = float(np.log(2.0) / P)
    invs2 = float(1.0 / np.sqrt(2.0))
    mscale = float(SCHR_MSCALE)

    n16 = len(M16_PAIRS) * 2     # k-blocks in m16
    n8 = len(M8_PAIRS) * 2       # k-blocks in m8

    nc = bacc.Bacc()
    qT = nc.declare_dram_parameter("qT", [bpc, P, s], bf16, isOutput=False)
    kT = nc.declare_dram_parameter("kT", [bpc, P, s], bf16, isOutput=False)
    # va host-packed as [p, kb*(D+1)]: row p holds v[kb*128+p, :]+[1] per kb
    va = nc.declare_dram_parameter(
        "va", [bpc, P, KB * (D + 1)], bf16, isOutput=False
    )
    # m16[b, i*2P+p, q] int16, i indexes M16_PAIRS: 0/1 not-mask rows for
    # E16 pairs
    m16 = nc.declare_dram_parameter("m16", [bpc, n16 * P, s], i16, isOutput=False)
    # m8: uint8 rows for SCHR pairs (bias codes {252 live, 24 dead}) and
    # EU8P pairs ({1, 0} multiplier), indexed by M8_PAIRS position
    m8 = nc.declare_dram_parameter("m8", [bpc, n8 * P, s], u8, isOutput=False)
    # out packed [b, qt, p, sl*D+d] so each partition row is one contiguous
    # 1KB chunk (avoids the <512B DMA latency penalty); host unpermutes
    out = nc.declare_dram_parameter(
        "out", [bpc, NQT, P, 4 * D], bf16, isOutput=True
    )

    KCW = [2 * P, 2 * P, 4 * P, 4 * P, 4 * P]
    KCO = [0, 2 * P, 4 * P, 8 * P, 12 * P]

    with tile.TileContext(nc) as tc:
        with (
            tc.tile_pool(name="qk", bufs=2) as qkp,
            tc.tile_pool(name="vp", bufs=2) as vp,
            tc.tile_pool(name="m16p", bufs=M16_BUFS) as m16p,
            tc.tile_pool(name="m8p", bufs=M8_BUFS) as m8p,
            tc.tile_pool(name="attn", bufs=ATT_BUFS) as attnp,
            tc.tile_pool(name="attne", bufs=ATTE_BUFS) as attnep,
            tc.tile_pool(name="schri", bufs=SCHRI_BUFS) as schrip,
            tc.tile_pool(name="outp", bufs=3) as outp,
            tc.tile_pool(name="const", bufs=1) as constp,
            tc.tile_pool(name="rp", bufs=12) as rp,
            tc.tile_pool(name="spsum", bufs=SPSUM_BUFS, space="PSUM") as spsum,
            tc.tile_pool(name="avpsum", bufs=AV_BUFS, space="PSUM") as avpsum,
        ):
            # ACT warm-up: load the Exp table (~1.3us) during initial DMAs
            warm = constp.tile([P, 128], bf16, name="warm", tag="warm")
            nc.vector.memset(warm[:], 0.0)
            nc.scalar.activation(
                warm[:, 0:1], warm[:, 0:1], mybir.ActivationFunctionType.Exp
            )
            # PE warm-up burst: p-state ramps to full clock with sustained
            # busy; burn the initial DMA-wait window on dummy matmuls
            wp = avpsum.tile([P, 2, D + 1], f32, name="warm_mm", tag="av")
            for _ in range(WARM_N):
                nc.tensor.matmul(
                    wp[:, 0, 0:P], lhsT=warm[:], rhs=warm[:],
                    start=True, stop=True,
                )

            # ---- per-batch tiles + loads, deadline-ordered -------------
            kt_ch = [[None] * len(KCW) for _ in range(bpc)]
            qt_ch = [[None] * NQT for _ in range(bpc)]
            va_s = [None] * bpc
            m16_t = [[None] * NQT for _ in range(bpc)]
            m8_t = [[None] * NQT for _ in range(bpc)]

            for b in range(bpc):
                kt_ch[b] = [
                    qkp.tile([P, w], bf16, name="kt_ch", tag=f"kt{i}", bufs=2)
                    for i, w in enumerate(KCW)
                ]
                qt_ch[b] = [
                    qkp.tile([P, QW], bf16, name="qt_ch", tag="qt",
                             bufs=NQT + 2)
                    for _ in range(NQT)
                ]
                va_s[b] = vp.tile([P, KB, D + 1], bf16, name="va_s", tag="va")
                m16_t[b] = [
                    m16p.tile([P, n16, QW], i16, name="m16_t", tag="m16",
                              bufs=M16_BUFS)
                    for _ in range(NQT)
                ]
                m8_t[b] = [
                    m8p.tile([P, n8, QW], u8, name="m8_t", tag="m8",
                             bufs=M8_BUFS)
                    for _ in range(NQT)
                ]

            def _ldk(b, i, eng):
                eng.dma_start(
                    out=kt_ch[b][i][:], in_=kT[b, :, KCO[i]:KCO[i] + KCW[i]]
                )

            def _ldq(b, qt, eng):
                eng.dma_start(
                    out=qt_ch[b][qt][:], in_=qT[b, :, qt * QW:(qt + 1) * QW]
                )

            def _ldva(b, lo, hi, eng):
                vr = va[b, :, :].rearrange("p (kb d) -> p kb d", d=D + 1)
                eng.dma_start(out=va_s[b][:, lo:hi, :], in_=vr[:, lo:hi, :])

            def _ldm16(b, qt, eng):
                m16r = m16[b, :, :].rearrange("(kb p) q -> p kb q", p=P)
                eng.dma_start(
                    out=m16_t[b][qt][:],
                    in_=m16r[:, :, qt * QW:(qt + 1) * QW],
                )

            def _ldm8(b, qt, eng, lo=0, hi=None):
                hi = n8 if hi is None else hi
                m8r = m8[b, :, :].rearrange("(kb p) q -> p kb q", p=P)
                eng.dma_start(
                    out=m8_t[b][qt][:, lo:hi, :],
                    in_=m8r[:, lo:hi, qt * QW:(qt + 1) * QW],
                )

            SP, ACT, POOL = nc.sync, nc.scalar, nc.gpsimd
            # batch 0, q-tile 0: masks split across ACT/Pool queues so the
            # first pass isn't gated on SP's ~650ns/DMA issue rate
            _ldk(0, 0, SP)
            _ldq(0, 0, SP)
            _ldm16(0, 0, SP)             # E16 pairs 0,7 (pair 0 first)
            _ldm8(0, 0, ACT, 0, 4)       # pairs 1,2
            _ldm8(0, 0, ACT, 4, 8)       # pairs 3,4
            _ldm8(0, 0, ACT, 8, 12)     # pairs 5,6
            _ldva(0, 0, 8, POOL)
            _ldva(0, 8, KB, POOL)
            _ldk(0, 1, SP)
            _ldk(0, 2, SP)
            _ldk(0, 3, SP)
            _ldk(0, 4, SP)
            for qt in range(1, NQT):
                _ldq(0, qt, SP)
                _ldm8(0, qt, SP)
                _ldm16(0, qt, SP)
            # batch 1: all on SP, deadline order
            _ldk(1, 0, SP)
            _ldq(1, 0, SP)
            _ldva(1, 0, 8, SP)
            _ldm8(1, 0, SP)
            _ldm16(1, 0, SP)
            _ldk(1, 1, SP)
            _ldva(1, 8, KB, SP)
            _ldk(1, 2, SP)
            _ldk(1, 3, SP)
            _ldk(1, 4, SP)
            for qt in range(1, NQT):
                _ldq(1, qt, SP)
                _ldm8(1, qt, SP)
                _ldm16(1, qt, SP)

            # ---- merged global pair stream -----------------------------
            def emit_norm(av_t, o_s, b_, qt_):
                for sl in range(4):
                    avap = av_t[sl // 2][:, sl % 2, :]
                    recip = rp.tile([P, 1], f32, name="recip", tag="recip")
                    nc.vector.reciprocal(recip[:], avap[:, D:D + 1])
                    nc.vector.tensor_scalar_mul(
                        o_s[:, sl, :], avap[:, 0:D], recip[:]
                    )
                nc.sync.dma_start(
                    out=out[b_, qt_, :, :].rearrange(
                        "p (sl d) -> p sl d", d=D
                    ),
                    in_=o_s[:, :, :],
                )

            TOT = bpc * NQT * NPAIR
            atts = [None] * TOT
            avs = {}    # pass index -> (av_t list, o_s)

            for g in range(TOT + PD):
                # AV block of step g-PD first: if QK(g) must wait on its
                # s_ps buffer (exp of g-SPSUM_BUFS still draining), the PE
                # does these matmuls instead of idling.
                gp = g - PD
                if gp >= 0:
                    bp = gp // (NQT * NPAIR)
                    tp = gp % NPAIR
                    pidx = gp // NPAIR
                    if tp == 0:
                        avs[pidx] = (
                            [
                                avpsum.tile(
                                    [P, 2, D + 1], f32, name="av", tag="av"
                                )
                                for _ in range(2)
                            ],
                            outp.tile([P, 4, D], bf16, name="o_s", tag="o_s"),
                        )
                    av_t, o_s = avs[pidx]
                    for h in range(2):
                        kb = 2 * tp + h
                        for sl in range(4):
                            nc.tensor.matmul(
                                av_t[sl // 2][:, sl % 2, :],
                                lhsT=atts[gp][:, h, sl * P:(sl + 1) * P],
                                rhs=va_s[bp][:, kb, :],
                                start=(kb == 0 and sl % 2 == 0),
                                stop=(kb == KB - 1 and sl % 2 == 1),
                            )
                    if tp == NPAIR - 1:
                        qtp = (gp // NPAIR) % NQT
                        emit_norm(av_t, o_s, bp, qtp)
                        del avs[pidx]

                if g < TOT:
                    b = g // (NQT * NPAIR)
                    qt = (g // NPAIR) % NQT
                    t = g % NPAIR
                    s_ps = spsum.tile([P, 2, QW], f32, name="s_ps", tag="s")
                    for h in range(2):
                        kb = 2 * t + h
                        kc = next(
                            i for i in range(len(KCW))
                            if KCO[i] <= kb * P < KCO[i] + KCW[i]
                        )
                        ko = kb * P - KCO[kc]
                        nc.tensor.matmul(
                            s_ps[:, h, :],
                            lhsT=kt_ch[b][kc][:, ko:ko + P],
                            rhs=qt_ch[b][qt][:],
                            start=True,
                            stop=True,
                        )
                    path = PATHS[t]
                    att = attnp.tile([P, 2, QW], bf16, name="att", tag="att")
                    if path == "SCHR":
                        pos = M8_POS[t]
                        att1 = schrip.tile(
                            [P, 2, QW], bf16, name="att1", tag="schri"
                        )
                        att2 = schrip.tile(
                            [P, 2, QW], bf16, name="att2", tag="schri"
                        )
                        # PSUM-reading ops stay on DVE (GPSIMD may not
                        # access PSUM); SBUF-only tail split DVE/Pool
                        nc.vector.scalar_tensor_tensor(
                            out=att1[:].bitcast(i16),
                            in0=m8_t[b][qt][:, 2 * pos:2 * pos + 2, :],
                            scalar=mscale,
                            in1=s_ps[:],
                            op0=mybir.AluOpType.mult,
                            op1=mybir.AluOpType.add,
                        )
                        nc.vector.tensor_scalar(
                            out=att2[:].bitcast(i16),
                            in0=att1[:].bitcast(i16),
                            scalar1=64,
                            scalar2=None,
                            op0=mybir.AluOpType.add,
                        )
                        nc.vector.tensor_scalar(
                            out=att2[:],
                            in0=att2[:],
                            scalar1=invs2,
                            scalar2=None,
                            op0=mybir.AluOpType.mult,
                        )
                        nc.gpsimd.tensor_add(att[:], att2[:], att1[:])
                    else:
                        att_e = attnep.tile(
                            [P, 2, QW], bf16, name="att_e", tag="atte"
                        )
                        nc.scalar.activation(
                            att_e[:],
                            s_ps[:],
                            mybir.ActivationFunctionType.Exp,
                            scale=escale,
                        )
                        if path == "E16":
                            pos = M16_POS[t]
                            nc.vector.tensor_mul(
                                att[:], att_e[:],
                                m16_t[b][qt][:, 2 * pos:2 * pos + 2, :],
                            )
                        else:  # EU8P
                            pos = M8_POS[t]
                            nc.gpsimd.tensor_mul(
                                att[:], att_e[:],
                                m8_t[b][qt][:, 2 * pos:2 * pos + 2, :],
                            )
                    atts[g] = att
    nc.compile()
    return nc


def _prep_inputs(q, k, v, mask):
    bf = ml_dtypes.bfloat16
    ones = np.ones((B, S, 1), dtype=np.float32)
    va_full = (
        np.concatenate([np.asarray(v, np.float32), ones], axis=2)
        .reshape(B, S // P, P, D + 1)
        .transpose(0, 2, 1, 3)
        .reshape(B, P, (S // P) * (D + 1))
        .astype(bf)
    )
    ascale = np.float32(P / np.log(2.0) / np.sqrt(D))
    qT_full = np.ascontiguousarray(
        np.asarray(q, np.float32).transpose(0, 2, 1) * ascale
    ).astype(bf)
    kT_full = np.ascontiguousarray(
        np.asarray(k, np.float32).transpose(0, 2, 1)
    ).astype(bf)
    mT = np.asarray(mask, bool).transpose(0, 2, 1)  # [b, k, q]
    m16_full = np.empty((B, len(M16_PAIRS) * 2 * P, S), dtype=np.int16)
    for i, pr in enumerate(M16_PAIRS):
        blk = mT[:, 2 * pr * P:(2 * pr + 2) * P, :]
        r0, r1 = 2 * i * P, (2 * i + 2) * P
        m16_full[:, r0:r1, :] = np.where(blk, np.int16(0), np.int16(1))
    m8_full = np.empty((B, len(M8_PAIRS) * 2 * P, S), dtype=np.uint8)
    for i, pr in enumerate(M8_PAIRS):
        blk = mT[:, 2 * pr * P:(2 * pr + 2) * P, :]
        r0, r1 = 2 * i * P, (2 * i + 2) * P
        if PATHS[pr] == "SCHR":
            m8_full[:, r0:r1, :] = np.where(
                blk, np.uint8(SCHR_M_DEAD), np.uint8(SCHR_M_LIVE)
            )
        else:
            m8_full[:, r0:r1, :] = (~blk).astype(np.uint8)
    return qT_full, kT_full, va_full, m16_full, m8_full


def kernel(q, k, v, mask, _trace=False, _trace_kwargs=None):
    global _NC, LAST_RESULT
    from concourse.bass_utils import run_bass_kernel_spmd

    if _NC is None:
        _NC = _build_nc()

    qT_full, kT_full, va_full, m16_full, m8_full = _prep_inputs(q, k, v, mask)

    in_maps = []
    for c in range(N_CORES):
        lo, hi = c * BPC, (c + 1) * BPC
        in_maps.append(
            {
                "qT": qT_full[lo:hi],
                "kT": kT_full[lo:hi],
                "va": va_full[lo:hi],
                "m16": m16_full[lo:hi],
                "m8": m8_full[lo:hi],
            }
        )

    kw = {}
    if _trace:
        kw["trace"] = True
        if _trace_kwargs:
            kw.update(_trace_kwargs)
    LAST_RESULT = run_bass_kernel_spmd(_NC, in_maps, list(range(N_CORES)), **kw)
    res = LAST_RESULT.results
    outp = np.concatenate(
        [np.asarray(res[c]["out"]) for c in range(N_CORES)], axis=0
    ).astype(np.float32)
    # [b, qt, p, sl*D+d] -> [b, qt*512 + sl*128 + p, d]
    out = (
        outp.reshape(B, NQT, P, 4, D)
        .transpose(0, 1, 3, 2, 4)
        .reshape(B, S, D)
    )
    return np.ascontiguousarray(out)
